# revision 1
# baseline (speedup 1.0000x reference)
"""Trainium2 Bass kernel for the 8-bit SNN barrel shifter.

Reference semantics (all inputs are exactly 0.0/1.0 f32):
    shift = S[:,0] + 2*S[:,1] + 4*S[:,2]
    out[:, i] = P[:, i - shift] if i >= shift else 0

Device strategy (pure data parallel over 8 cores, row-major layout):
  - host repacks P/S to uint8 bits (0/1) and shards rows across the 8 cores
  - per core the vector engine packs each row's 8 bit-bytes into one packed
    byte with a bitwise OR-tree over uint32 views (junk bits tracked >= 8),
    packs the 3 shift bits, applies one per-element logical_shift_left, and
    extracts bit pairs with single shift ops (one uint16 lane per 2 output
    bytes; each output byte holds its bit at a known position)
  - host re-interleaves the pair planes, masks the known junk bits, and
    casts back to f32
"""
import numpy as np

_N = 4194304
_CORES = 8
_NC = _N // _CORES          # rows per core
_PARTS = 128
_R = (512, 1024, 1024, 1024, 512)  # per-tile rows-per-partition schedule
# tile count follows the _R schedule
_POOL_PAIRS = 0             # how many of the 4 pair-extract ops go to GpSimd

_CACHE: dict = {}


def _build(rows_per_core: int, R, pool_pairs: int = _POOL_PAIRS, bufs: int = 3):
    import concourse.tile as tile
    from concourse import bacc, mybir

    dt = mybir.dt
    Alu = mybir.AluOpType
    P = _PARTS
    rpp = rows_per_core // P          # rows per partition
    rs = [R] * (rpp // R) if isinstance(R, int) else list(R)
    assert sum(rs) == rpp

    nc = bacc.Bacc("TRN2", target_bir_lowering=False, debug=False)
    p8 = nc.dram_tensor("p8", (rows_per_core, 8), dt.uint8, kind="ExternalInput").ap()
    s8 = nc.dram_tensor("s8", (rows_per_core, 4), dt.uint8, kind="ExternalInput").ap()
    o16 = nc.dram_tensor("o16", (rows_per_core * 4,), dt.uint16,
                         kind="ExternalOutput").ap()

    pr = p8.rearrange("(p r) c -> p r c", p=P, r=rpp)
    sr = s8.rearrange("(p r) c -> p r c", p=P, r=rpp)

    with tile.TileContext(nc) as tc:
        with tc.tile_pool(name="io", bufs=bufs) as io, tc.tile_pool(name="tmp", bufs=2) as tmp:
            r0 = 0
            for R in rs:
                pt = io.tile([P, R, 8], dt.uint8, tag="p")
                st = io.tile([P, R, 4], dt.uint8, tag="s")
                nc.sync.dma_start(pt[:], pr[:, r0:r0 + R])
                nc.sync.dma_start(st[:], sr[:, r0:r0 + R])

                # host sends P columns permuted [0,2,4,6,1,3,5,7], so the two
                # uint32 views hold even bits / odd bits at byte positions.
                # Fold tree (junk tracked; bits 0..7 of the low half are the
                # packed byte):
                #   m = x32_odd<<1 | x32_even -> pairs at {0,1},{8,9},{16,17},{24,25}
                #   n = m>>6 | m              -> quads at {0..3}, {16..19}
                #   vi32 = n>>12 | n          -> byte at {0..7}, junk 8..13, >=16
                x32 = pt[:].bitcast(dt.uint32)          # [P, R, 2]
                m = tmp.tile([P, R], dt.uint32, tag="m")
                nc.vector.scalar_tensor_tensor(
                    m[:], x32[:, :, 1], 1, x32[:, :, 0],
                    op0=Alu.logical_shift_left, op1=Alu.bitwise_or)
                n = tmp.tile([P, R], dt.uint32, tag="n")
                nc.vector.scalar_tensor_tensor(
                    n[:], m[:], 6, m[:],
                    op0=Alu.logical_shift_right, op1=Alu.bitwise_or)
                # final fold on uint16 views of n: even halves hold the low
                # quad, odd halves the high quad -> dense uint16 vi
                n16 = n[:].bitcast(dt.uint16)           # [P, 2R]
                vi = tmp.tile([P, R], dt.uint16, tag="vi")
                nc.vector.scalar_tensor_tensor(
                    vi[:], n16[:, 1::2], 4, n16[:, 0::2],
                    op0=Alu.logical_shift_left, op1=Alu.bitwise_or)

                # pack S bits: ti = s0 + 2*s1 + 4*s2
                a = tmp.tile([P, R], dt.uint8, tag="a")
                nc.vector.scalar_tensor_tensor(
                    a[:], st[:, :, 2], 1, st[:, :, 1],
                    op0=Alu.logical_shift_left, op1=Alu.bitwise_or)
                ti = tmp.tile([P, R], dt.uint16, tag="ti")
                nc.vector.scalar_tensor_tensor(
                    ti[:], a[:], 2, st[:, :, 0],
                    op0=Alu.mult, op1=Alu.add)

                # vs = vi << ti (per-element shift, uint16)
                vs = tmp.tile([P, R], dt.uint16, tag="vs")
                nc.vector.tensor_tensor(vs[:], vi[:], ti[:], op=Alu.logical_shift_left)

                # extract bit pairs: lane k holds bit 2k at byte0.bit7 and
                # bit 2k+1 at byte1.bit0 (junk elsewhere, host masks)
                ot = io.tile([P, 4, R], dt.uint16, tag="o")
                for k in range(4):
                    eng = nc.gpsimd if k < pool_pairs else nc.vector
                    eng.tensor_scalar(
                        ot[:, k, :], vs[:], 7 - 2 * k, None,
                        op0=Alu.logical_shift_left)

                dst = o16[4 * P * r0: 4 * P * (r0 + R)].rearrange(
                    "(p c r) -> p c r", p=P, c=4, r=R)
                nc.scalar.dma_start(dst, ot[:])
                r0 += R
    nc.compile()
    _fix_bitwise_imms(nc, mybir)
    return nc


_BITWISE = None


def _fix_bitwise_imms(nc, mybir):
    """walrus requires integer immediates matching the src dtype on bitvec
    tensor_scalar ops; bass emits float32/int32 — rewrite them."""
    global _BITWISE
    Alu = mybir.AluOpType
    if _BITWISE is None:
        _BITWISE = {
            Alu.bitwise_and, Alu.bitwise_or, Alu.bitwise_xor, Alu.bitwise_not,
            Alu.logical_shift_left, Alu.logical_shift_right,
            Alu.arith_shift_left, Alu.arith_shift_right,
        }
    for f in nc.m.functions:
        for blk in f.blocks:
            for i in blk.instructions:
                if type(i).__name__ != "InstTensorScalarPtr":
                    continue
                ops = [getattr(i, "op0", None), getattr(i, "op1", None)]
                if not any(op in _BITWISE for op in ops if op is not None):
                    continue
                src_dt = i.ins[0].dtype
                for k in range(1, len(i.ins)):
                    iv = i.ins[k]
                    if isinstance(iv, mybir.ImmediateValue):
                        i.ins[k] = mybir.ImmediateValue(
                            dtype=src_dt, value=int(iv.value))


def _get_nc():
    key = (_NC, tuple(_R) if not isinstance(_R, int) else _R)
    if key not in _CACHE:
        _CACHE[key] = _build(*key)
    return _CACHE[key]


_PERM = [0, 2, 4, 6, 1, 3, 5, 7]


def _prep_inputs(P, S):
    Pb = np.ascontiguousarray(np.asarray(P, dtype=np.float32)[:, _PERM]).astype(np.uint8)
    s8 = np.zeros((P.shape[0], 4), np.uint8)
    s8[:, :3] = np.ascontiguousarray(S).astype(np.uint8)
    return Pb, s8


def _unshard_core(o16, rows_per_core, R):
    P = _PARTS
    rpp = rows_per_core // P
    rs = [R] * (rpp // R) if isinstance(R, int) else list(R)
    rows = np.empty((P, rpp, 8), np.uint8)
    r0 = 0
    for Rt in rs:
        chunk = o16[4 * P * r0: 4 * P * (r0 + Rt)].reshape(P, 4, Rt)
        b = chunk.view(np.uint8).reshape(P, 4, Rt, 2)
        rows[:, r0:r0 + Rt, 0::2] = ((b[..., 0] >> 7) & 1).transpose(0, 2, 1)
        rows[:, r0:r0 + Rt, 1::2] = (b[..., 1] & 1).transpose(0, 2, 1)
        r0 += Rt
    return rows.reshape(rows_per_core, 8)


def _unshard_out(o16_list):
    out = np.empty((_N, 8), np.float32)
    for c, r in enumerate(o16_list):
        out[c * _NC:(c + 1) * _NC] = _unshard_core(r.ravel(), _NC, _R)
    return out


def kernel(P: np.ndarray, S: np.ndarray) -> np.ndarray:
    from concourse.bass_utils import run_bass_kernel_spmd

    nc = _get_nc()
    Pb, s8 = _prep_inputs(P, S)
    in_maps = [
        {"p8": Pb[c * _NC:(c + 1) * _NC], "s8": s8[c * _NC:(c + 1) * _NC]}
        for c in range(_CORES)
    ]
    res = run_bass_kernel_spmd(nc, in_maps, core_ids=list(range(_CORES)))
    return _unshard_out([r["o16"] for r in res.results])



# revision 2
# speedup vs baseline: 2.3350x; 2.3350x over previous
"""Trainium2 Bass kernel for the 8-bit SNN barrel shifter.

Reference semantics (all inputs are exactly 0.0/1.0 f32):
    shift = S[:,0] + 2*S[:,1] + 4*S[:,2]
    out[:, i] = P[:, i - shift] if i >= shift else 0

Equivalently, packing row bits little-endian into one byte v and the
shift amount into t in 0..7:  out_byte = (v << t) & 0xFF.

Device strategy (pure data parallel over 8 cores, row-major layout):
  - host packs P rows to one byte (np.packbits) and S to a 0..7 byte,
    shards rows contiguously across the 8 cores
  - per core the vector engine computes one u8 tensor_tensor
    logical_shift_left per element (verified on HW: u8 shift wraps,
    bits >=8 drop), output is one byte per row
  - host unpacks bits back to f32
Device IO is 3 bytes/row (1.5 MB/core) vs 20 bytes/row for the naive
packing; the kernel is DMA-bound.
"""
import numpy as np

_N = 4194304
_CORES = 8
_NC = _N // _CORES          # rows per core
_PARTS = 128
_R = (512, 1024, 1024, 1024, 512)  # per-tile rows-per-partition schedule
_BUFS = 3

_CACHE: dict = {}


def _build(rows_per_core: int, R, bufs: int = _BUFS):
    import concourse.tile as tile
    from concourse import bacc, mybir

    dt = mybir.dt
    Alu = mybir.AluOpType
    P = _PARTS
    rpp = rows_per_core // P          # rows (bytes) per partition
    rs = [R] * (rpp // R) if isinstance(R, int) else list(R)
    assert sum(rs) == rpp

    nc = bacc.Bacc("TRN2", target_bir_lowering=False, debug=False)
    vi_d = nc.dram_tensor("vi", (rows_per_core,), dt.uint8, kind="ExternalInput").ap()
    ti_d = nc.dram_tensor("ti", (rows_per_core,), dt.uint8, kind="ExternalInput").ap()
    o_d = nc.dram_tensor("o", (rows_per_core,), dt.uint8, kind="ExternalOutput").ap()

    vr = vi_d.rearrange("(p r) -> p r", p=P, r=rpp)
    tr = ti_d.rearrange("(p r) -> p r", p=P, r=rpp)
    orr = o_d.rearrange("(p r) -> p r", p=P, r=rpp)

    with tile.TileContext(nc) as tc:
        with tc.tile_pool(name="io", bufs=bufs) as io:
            r0 = 0
            for j, R in enumerate(rs):
                vt = io.tile([P, R], dt.uint8, tag="v")
                tt = io.tile([P, R], dt.uint8, tag="t")
                nc.sync.dma_start(vt[:], vr[:, r0:r0 + R])
                nc.scalar.dma_start(tt[:], tr[:, r0:r0 + R])

                ot = io.tile([P, R], dt.uint8, tag="o")
                nc.vector.tensor_tensor(ot[:], vt[:], tt[:],
                                        op=Alu.logical_shift_left)

                eng = nc.sync if j % 2 == 0 else nc.scalar
                eng.dma_start(orr[:, r0:r0 + R], ot[:])
                r0 += R
    nc.compile()
    return nc


def _get_nc():
    key = (_NC, tuple(_R) if not isinstance(_R, int) else _R, _BUFS)
    if key not in _CACHE:
        _CACHE[key] = _build(*key)
    return _CACHE[key]


def _prep_inputs(P, S):
    Pb = np.asarray(P, dtype=np.float32).astype(np.uint8)
    vi = np.packbits(Pb, axis=1, bitorder="little").ravel()
    Sb = np.asarray(S, dtype=np.float32).astype(np.uint8)
    ti = (Sb[:, 0] | (Sb[:, 1] << 1) | (Sb[:, 2] << 2)).astype(np.uint8)
    return vi, ti


def _in_maps(P, S):
    vi, ti = _prep_inputs(P, S)
    return [
        {"vi": vi[c * _NC:(c + 1) * _NC], "ti": ti[c * _NC:(c + 1) * _NC]}
        for c in range(_CORES)
    ]


def _unshard_out(o_list):
    o8 = np.concatenate([r.ravel() for r in o_list])
    return np.unpackbits(o8[:, None], axis=1, bitorder="little").astype(np.float32)


def kernel(P: np.ndarray, S: np.ndarray) -> np.ndarray:
    from concourse.bass_utils import run_bass_kernel_spmd

    nc = _get_nc()
    res = run_bass_kernel_spmd(nc, _in_maps(P, S), core_ids=list(range(_CORES)))
    return _unshard_out([r["o"] for r in res.results])


# revision 3
# speedup vs baseline: 3.5844x; 1.5351x over previous
"""Trainium2 Bass kernel for the 8-bit SNN barrel shifter.

Reference semantics (all inputs are exactly 0.0/1.0 f32):
    shift = S[:,0] + 2*S[:,1] + 4*S[:,2]
    out[:, i] = P[:, i - shift] if i >= shift else 0

Packing row bits little-endian into one byte v and the shift amount
into t in 0..7, the row result is (v << t) & 0xFF. We compute it
bit-reversed: with vr = bitrev8(v), the device evaluates vr >> t in
uint16 (a logical right shift drops the shifted-out bits, so the
result is exactly the bit-reversed answer, always <= 255), and the
host reverses bits back.

Device program (hand-scheduled raw bass, data parallel over 8 cores):
  - both u16 inputs are preloaded into SBUF via one HWDGE DMA per
    queue (qSP / qAct)
  - DVE computes the per-row u16 logical_shift_right in 4 tiles
    (16-bit ops run ~2x faster per element than 8-bit on DVE)
  - per tile, a gpsimd SWDGE DMA writes the result to DRAM, casting
    u16 -> u8 during the transfer (values <= 255, so the cast is exact)
  - gpsimd waits for the out DMAs and drains; the runtime epilogue
    handles semaphore cleanup
Host does only format conversion: f32 0/1 <-> packed bits (packbits /
unpackbits / bit-reverse LUT), all vectorized numpy.
"""
import numpy as np

_N = 4194304
_CORES = 8
_NC = _N // _CORES          # rows per core
_P = 128                    # SBUF partitions
_RPP = _NC // _P            # rows per partition (4096)
_SIZES = (1024, 1024, 1024, 1024)

_CACHE: dict = {}

# bit-reverse LUT, value domain u16 so inputs upload zero-extended
_REV = np.array([int(f"{i:08b}"[::-1], 2) for i in range(256)],
                dtype=np.uint16)
_REV8 = _REV.astype(np.uint8)


def _build(sizes=_SIZES):
    from concourse import bacc, mybir

    dt = mybir.dt
    Alu = mybir.AluOpType
    P, RPP = _P, _RPP
    assert sum(sizes) == RPP

    nc = bacc.Bacc("TRN2", target_bir_lowering=False, debug=False)
    vi_d = nc.dram_tensor("vi", (P * RPP,), dt.uint16, kind="ExternalInput").ap()
    ti_d = nc.dram_tensor("ti", (P * RPP,), dt.uint16, kind="ExternalInput").ap()
    o_d = nc.dram_tensor("o", (P * RPP,), dt.uint8, kind="ExternalOutput").ap()
    vr = vi_d.rearrange("(p r) -> p r", p=P, r=RPP)
    tr = ti_d.rearrange("(p r) -> p r", p=P, r=RPP)
    orr = o_d.rearrange("(p r) -> p r", p=P, r=RPP)

    s_vi = nc.alloc_semaphore("s_vi")
    s_ti = nc.alloc_semaphore("s_ti")
    s_c = nc.alloc_semaphore("s_c")
    s_od = nc.alloc_semaphore("s_od")

    with (
        nc.sbuf_tensor("vb", [P, RPP], dt.uint16) as vb,
        nc.sbuf_tensor("tb", [P, RPP], dt.uint16) as tb,
        nc.sbuf_tensor("ob", [P, RPP], dt.uint16) as ob,
    ):
        nc.sync.dma_start(vb[:], vr[:]).then_inc(s_vi, 16)
        nc.scalar.dma_start(tb[:], tr[:]).then_inc(s_ti, 16)

        nc.vector.wait_ge(s_vi, 16)
        nc.vector.wait_ge(s_ti, 16)
        r0 = 0
        for j, r in enumerate(sizes):
            sl = slice(r0, r0 + r)
            nc.vector.tensor_tensor(
                ob[:, sl], vb[:, sl], tb[:, sl], op=Alu.logical_shift_right
            ).then_inc(s_c, 1)
            r0 += r

        r0 = 0
        for j, r in enumerate(sizes):
            sl = slice(r0, r0 + r)
            nc.gpsimd.wait_ge(s_c, j + 1)
            nc.gpsimd.dma_start(orr[:, sl], ob[:, sl]).then_inc(s_od, 16)
            r0 += r

        nc.gpsimd.wait_ge(s_od, 16 * len(sizes))
        nc.gpsimd.drain()

    nc.compile()
    _strip_const_memsets(nc)
    return nc


def _strip_const_memsets(nc):
    """Drop the dead const-tensor Memsets Bacc emits at init (they are
    never read; removing them keeps the program lean)."""
    for f in nc.m.functions:
        for blk in f.blocks:
            keep = [i for i in blk.instructions
                    if not (type(i).__name__ == "InstMemset" and i.outs and
                            getattr(i.outs[0], "memref", "").startswith("const-"))]
            if len(keep) != len(blk.instructions):
                blk.instructions[:] = keep


def _get_nc():
    key = tuple(_SIZES)
    if key not in _CACHE:
        _CACHE[key] = _build(key)
    return _CACHE[key]


def _prep_inputs(P, S):
    Pb = np.asarray(P, dtype=np.float32).astype(np.uint8)
    v8 = np.packbits(Pb, axis=1, bitorder="little").ravel()
    vrev = _REV[v8]                                   # u16, bit-reversed
    Sb = np.asarray(S, dtype=np.float32).astype(np.uint8)
    ti = (Sb[:, 0] | (Sb[:, 1] << 1) | (Sb[:, 2] << 2)).astype(np.uint16)
    return vrev, ti


def _in_maps(P, S):
    vrev, ti = _prep_inputs(P, S)
    return [
        {"vi": vrev[c * _NC:(c + 1) * _NC], "ti": ti[c * _NC:(c + 1) * _NC]}
        for c in range(_CORES)
    ]


def _unshard_out(o_list):
    orev = np.concatenate([r.ravel() for r in o_list])   # bit-reversed bytes
    o8 = _REV8[orev]
    return np.unpackbits(o8[:, None], axis=1, bitorder="little").astype(np.float32)


def kernel(P: np.ndarray, S: np.ndarray) -> np.ndarray:
    from concourse.bass_utils import run_bass_kernel_spmd

    nc = _get_nc()
    res = run_bass_kernel_spmd(nc, _in_maps(P, S), core_ids=list(range(_CORES)))
    return _unshard_out([r["o"] for r in res.results])


# revision 4
# speedup vs baseline: 3.6882x; 1.0290x over previous
"""Trainium2 Bass kernel for the 8-bit SNN barrel shifter.

Reference semantics (all inputs are exactly 0.0/1.0 f32):
    shift = S[:,0] + 2*S[:,1] + 4*S[:,2]
    out[:, i] = P[:, i - shift] if i >= shift else 0

Packing row bits little-endian into one byte v and the shift amount
into t in 0..7, the row result is (v << t) & 0xFF. We compute it
bit-reversed: with vr = bitrev8(v), the device evaluates vr >> t in
uint16 (a logical right shift drops the shifted-out bits, so the
result is exactly the bit-reversed answer, always <= 255), and the
host reverses bits back.

Device program (hand-scheduled raw bass, data parallel over 8 cores):
  - both u16 inputs are preloaded into SBUF via one HWDGE DMA per
    queue (qSP / qAct)
  - DVE computes the per-row u16 logical_shift_right in 4 tiles
    (16-bit ops run ~2x faster per element than 8-bit on DVE)
  - per tile, the result streams to DRAM as dense u16 on the two
    HWDGE queues (alternating); the host keeps the low byte
  - each queue engine waits for its out DMAs and drains; the runtime
    epilogue handles semaphore cleanup
Host does only format conversion: f32 0/1 <-> packed bits (packbits /
unpackbits / bit-reverse LUT), all vectorized numpy.
"""
import numpy as np

_N = 4194304
_CORES = 8
_NC = _N // _CORES          # rows per core
_P = 128                    # SBUF partitions
_RPP = _NC // _P            # rows per partition (4096)
_SIZES = (1024, 1024, 1024, 1024)

_CACHE: dict = {}

# bit-reverse LUT, value domain u16 so inputs upload zero-extended
_REV = np.array([int(f"{i:08b}"[::-1], 2) for i in range(256)],
                dtype=np.uint16)
_REV8 = _REV.astype(np.uint8)


def _build(sizes=_SIZES):
    from concourse import bacc, mybir

    dt = mybir.dt
    Alu = mybir.AluOpType
    P, RPP = _P, _RPP
    assert sum(sizes) == RPP

    nc = bacc.Bacc("TRN2", target_bir_lowering=False, debug=False)
    vi_d = nc.dram_tensor("vi", (P * RPP,), dt.uint16, kind="ExternalInput").ap()
    ti_d = nc.dram_tensor("ti", (P * RPP,), dt.uint16, kind="ExternalInput").ap()
    o_d = nc.dram_tensor("o", (P * RPP,), dt.uint16, kind="ExternalOutput").ap()
    vr = vi_d.rearrange("(p r) -> p r", p=P, r=RPP)
    tr = ti_d.rearrange("(p r) -> p r", p=P, r=RPP)
    orr = o_d.rearrange("(p r) -> p r", p=P, r=RPP)

    s_vi = nc.alloc_semaphore("s_vi")
    s_ti = nc.alloc_semaphore("s_ti")
    s_c = nc.alloc_semaphore("s_c")
    s_osp = nc.alloc_semaphore("s_osp")
    s_oact = nc.alloc_semaphore("s_oact")

    with (
        nc.sbuf_tensor("vb", [P, RPP], dt.uint16) as vb,
        nc.sbuf_tensor("tb", [P, RPP], dt.uint16) as tb,
        nc.sbuf_tensor("ob", [P, RPP], dt.uint16) as ob,
    ):
        nc.sync.dma_start(vb[:], vr[:]).then_inc(s_vi, 16)
        nc.scalar.dma_start(tb[:], tr[:]).then_inc(s_ti, 16)

        nc.vector.wait_ge(s_vi, 16)
        nc.vector.wait_ge(s_ti, 16)
        r0 = 0
        for j, r in enumerate(sizes):
            sl = slice(r0, r0 + r)
            nc.vector.tensor_tensor(
                ob[:, sl], vb[:, sl], tb[:, sl], op=Alu.logical_shift_right
            ).then_inc(s_c, 1)
            r0 += r

        r0 = 0
        n_sp = n_act = 0
        for j, r in enumerate(sizes):
            sl = slice(r0, r0 + r)
            if j % 2 == 0:
                eng, sem = nc.sync, s_osp
                n_sp += 1
            else:
                eng, sem = nc.scalar, s_oact
                n_act += 1
            eng.wait_ge(s_c, j + 1)
            eng.dma_start(orr[:, sl], ob[:, sl]).then_inc(sem, 16)
            r0 += r

        nc.sync.wait_ge(s_osp, 16 * n_sp)
        nc.sync.drain()
        if n_act:
            nc.scalar.wait_ge(s_oact, 16 * n_act)
            nc.scalar.drain()

    nc.compile()
    _strip_const_memsets(nc)
    return nc


def _strip_const_memsets(nc):
    """Drop the dead const-tensor Memsets Bacc emits at init (they are
    never read; removing them keeps the program lean)."""
    for f in nc.m.functions:
        for blk in f.blocks:
            keep = [i for i in blk.instructions
                    if not (type(i).__name__ == "InstMemset" and i.outs and
                            getattr(i.outs[0], "memref", "").startswith("const-"))]
            if len(keep) != len(blk.instructions):
                blk.instructions[:] = keep


def _get_nc():
    key = tuple(_SIZES)
    if key not in _CACHE:
        _CACHE[key] = _build(key)
    return _CACHE[key]


def _prep_inputs(P, S):
    Pb = np.asarray(P, dtype=np.float32).astype(np.uint8)
    v8 = np.packbits(Pb, axis=1, bitorder="little").ravel()
    vrev = _REV[v8]                                   # u16, bit-reversed
    Sb = np.asarray(S, dtype=np.float32).astype(np.uint8)
    ti = (Sb[:, 0] | (Sb[:, 1] << 1) | (Sb[:, 2] << 2)).astype(np.uint16)
    return vrev, ti


def _in_maps(P, S):
    vrev, ti = _prep_inputs(P, S)
    return [
        {"vi": vrev[c * _NC:(c + 1) * _NC], "ti": ti[c * _NC:(c + 1) * _NC]}
        for c in range(_CORES)
    ]


def _unshard_out(o_list):
    o16 = np.concatenate([r.ravel() for r in o_list])    # u16, low byte valid
    orev = o16.view(np.uint8)[0::2]                      # bit-reversed bytes
    o8 = _REV8[orev]
    return np.unpackbits(np.ascontiguousarray(o8)[:, None], axis=1,
                         bitorder="little").astype(np.float32)


def kernel(P: np.ndarray, S: np.ndarray) -> np.ndarray:
    from concourse.bass_utils import run_bass_kernel_spmd

    nc = _get_nc()
    res = run_bass_kernel_spmd(nc, _in_maps(P, S), core_ids=list(range(_CORES)))
    return _unshard_out([r["o"] for r in res.results])


# revision 5
# speedup vs baseline: 4.1173x; 1.1163x over previous
"""Trainium2 Bass kernel for the 8-bit SNN barrel shifter.

Reference semantics (all inputs are exactly 0.0/1.0 f32):
    shift = S[:,0] + 2*S[:,1] + 4*S[:,2]
    out[:, i] = P[:, i - shift] if i >= shift else 0

Packing row bits little-endian into one byte v and the shift into t in
0..7, the row result is (v << t) & 0xFF. Computed bit-reversed as a
logical RIGHT shift (shifted-out bits vanish, result <= 255).

Quad packing: the host groups FOUR rows sharing the same t into one
u32 word w = ra | rb<<8 | rc<<16 | rd<<24 (r* = bitrev8 of the row
byte). The device computes w >> t in one u32 op: each byte field then
holds its row's reversed result in its low 8-t bits; the neighbor's
spill lands only in the top t bits, which are provably zero in the
true result, so the host masks with 0xFF >> t. This quarters the
vector-engine element count and reaches the 1-byte-per-row output
minimum on the fast hardware DMA queues.

Device program (hand-scheduled raw bass, data parallel over 8 cores):
  - u32 quad inputs preloaded via one HWDGE DMA per queue (qSP/qAct)
  - DVE: 4 tiled u32 logical_shift_right ops
  - per tile the u32 result streams out on the two HWDGE queues
  - each queue engine waits for its outs and drains; the runtime
    epilogue handles semaphore cleanup
Host does only data marshalling: f32 0/1 <-> packed bits, bit-reverse
LUT, equal-shift grouping (argsort) and the inverse scatter.
"""
import numpy as np

_N = 4194304
_CORES = 8
_NC = _N // _CORES          # rows per core
_P = 128                    # SBUF partitions
_RPPQ = 1025                # u32 quads per partition (padded)
_NQ = _P * _RPPQ            # quads per core (>= _NC/4 + pad)
_SIZES = (257, 256, 256, 256)

_CACHE: dict = {}

_REV8 = np.array([int(f"{i:08b}"[::-1], 2) for i in range(256)],
                 dtype=np.uint8)
_MASK = (0xFF >> np.arange(8)).astype(np.uint8)


def _build(sizes=_SIZES):
    from concourse import bacc, mybir

    dt = mybir.dt
    Alu = mybir.AluOpType
    P, RPPQ = _P, _RPPQ
    assert sum(sizes) == RPPQ

    nc = bacc.Bacc("TRN2", target_bir_lowering=False, debug=False)
    vi_d = nc.dram_tensor("vi", (_NQ,), dt.uint32, kind="ExternalInput").ap()
    ti_d = nc.dram_tensor("ti", (_NQ,), dt.uint32, kind="ExternalInput").ap()
    o_d = nc.dram_tensor("o", (_NQ,), dt.uint32, kind="ExternalOutput").ap()
    vr = vi_d.rearrange("(p r) -> p r", p=P, r=RPPQ)
    tr = ti_d.rearrange("(p r) -> p r", p=P, r=RPPQ)
    orr = o_d.rearrange("(p r) -> p r", p=P, r=RPPQ)

    s_vi = nc.alloc_semaphore("s_vi")
    s_ti = nc.alloc_semaphore("s_ti")
    s_c = nc.alloc_semaphore("s_c")
    s_osp = nc.alloc_semaphore("s_osp")
    s_oact = nc.alloc_semaphore("s_oact")

    with (
        nc.sbuf_tensor("vb", [P, RPPQ], dt.uint32) as vb,
        nc.sbuf_tensor("tb", [P, RPPQ], dt.uint32) as tb,
        nc.sbuf_tensor("ob", [P, RPPQ], dt.uint32) as ob,
    ):
        nc.sync.dma_start(vb[:], vr[:]).then_inc(s_vi, 16)
        nc.scalar.dma_start(tb[:], tr[:]).then_inc(s_ti, 16)

        nc.vector.wait_ge(s_vi, 16)
        nc.vector.wait_ge(s_ti, 16)
        r0 = 0
        for j, r in enumerate(sizes):
            sl = slice(r0, r0 + r)
            nc.vector.tensor_tensor(
                ob[:, sl], vb[:, sl], tb[:, sl], op=Alu.logical_shift_right
            ).then_inc(s_c, 1)
            r0 += r

        r0 = 0
        n_sp = n_act = 0
        for j, r in enumerate(sizes):
            sl = slice(r0, r0 + r)
            if j % 2 == 0:
                eng, sem = nc.sync, s_osp
                n_sp += 1
            else:
                eng, sem = nc.scalar, s_oact
                n_act += 1
            eng.wait_ge(s_c, j + 1)
            eng.dma_start(orr[:, sl], ob[:, sl]).then_inc(sem, 16)
            r0 += r

        nc.sync.wait_ge(s_osp, 16 * n_sp)
        nc.sync.drain()
        if n_act:
            nc.scalar.wait_ge(s_oact, 16 * n_act)
            nc.scalar.drain()

    nc.compile()
    _strip_const_memsets(nc)
    return nc


def _strip_const_memsets(nc):
    """Drop the dead const-tensor Memsets Bacc emits at init (they are
    never read; removing them keeps the program lean)."""
    for f in nc.m.functions:
        for blk in f.blocks:
            keep = [i for i in blk.instructions
                    if not (type(i).__name__ == "InstMemset" and i.outs and
                            getattr(i.outs[0], "memref", "").startswith("const-"))]
            if len(keep) != len(blk.instructions):
                blk.instructions[:] = keep


def _get_nc():
    key = tuple(_SIZES)
    if key not in _CACHE:
        _CACHE[key] = _build(key)
    return _CACHE[key]


def _pack_core(v8, t8):
    """Group 4 equal-t rows into u32 quads for one core's shard."""
    order = np.argsort(t8, kind="stable").astype(np.int32)
    ts = t8[order]
    idx_list = []
    for t in range(8):
        sel = order[ts == t]
        pad = (-len(sel)) % 4
        if pad:
            sel = np.concatenate([sel, np.full(pad, -1, np.int32)])
        idx_list.append(sel.reshape(-1, 4))
    idx = np.concatenate(idx_list)
    full = np.full((_NQ, 4), -1, np.int32)
    full[:len(idx)] = idx
    vr = _REV8[v8]
    wb = np.where(full >= 0, vr[np.clip(full, 0, None)], 0).astype(np.uint8)
    w = np.ascontiguousarray(wb).view(np.uint32).ravel()
    tq = np.where(full[:, 0] >= 0, t8[np.clip(full[:, 0], 0, None)], 0)
    return w, tq.astype(np.uint32), full


def _prep(P, S):
    Pb = np.asarray(P, dtype=np.float32).astype(np.uint8)
    v8 = np.packbits(Pb, axis=1, bitorder="little").ravel()
    Sb = np.asarray(S, dtype=np.float32).astype(np.uint8)
    t8 = (Sb[:, 0] | (Sb[:, 1] << 1) | (Sb[:, 2] << 2)).astype(np.uint8)
    return [_pack_core(v8[c * _NC:(c + 1) * _NC], t8[c * _NC:(c + 1) * _NC])
            for c in range(_CORES)]


def _in_maps(P, S):
    return [{"vi": p[0], "ti": p[1]} for p in _prep(P, S)]


def _unpack_core(w_out, tq, idx):
    b = w_out.view(np.uint8).reshape(-1, 4)
    res = b & _MASK[tq.astype(np.uint8)][:, None]
    out8 = np.empty(_NC, np.uint8)
    valid = idx >= 0
    out8[idx[valid]] = _REV8[res[valid]]
    return out8


def kernel(P: np.ndarray, S: np.ndarray) -> np.ndarray:
    from concourse.bass_utils import run_bass_kernel_spmd

    nc = _get_nc()
    packs = _prep(P, S)
    in_maps = [{"vi": p[0], "ti": p[1]} for p in packs]
    res = run_bass_kernel_spmd(nc, in_maps, core_ids=list(range(_CORES)))
    o8 = np.concatenate([
        _unpack_core(res.results[c]["o"].ravel(), packs[c][1], packs[c][2])
        for c in range(_CORES)])
    return np.unpackbits(o8[:, None], axis=1,
                         bitorder="little").astype(np.float32)
